# revision 12
# baseline (speedup 1.0000x reference)
"""Trainium2 Bass kernel for CRF Viterbi decode (nn_CRFLayer).

Strategy (pure data parallel over batch, per sharding hint):
- Host: sort batch rows by sequence length (desc), deal round-robin to the
  8 cores so every core gets an near-identical active-row schedule.
- Device (per core, 16 rows): the O(B*T*N^2) forward max-plus scan.
  Layout: scores_T[j_part, i_free] = transT[j, i] + alpha[i], computed as a
  single fused DVE tensor_tensor_reduce (add + max-reduce) per (row, j-tile).
  alpha (a 256-vector per row) is replicated across partitions by a rank-1
  PE matmul (ones x alpha) into PSUM; the per-step max outputs are
  transposed back to row-major by PE, the potentials added by an
  accumulating PE matmul (identity x pot), and ACT evicts PSUM->SBUF.
  Rows whose sequence ended are simply not computed (program is specialized
  on the active-row counts derived from sequence_lengths).
- Device streams out m_pre[t] = max_i(alpha_{t-1}[i] + trans[i, j]) per step.
- Host: reconstructs alpha_t = m_pre[t] + pot_t exactly (same single f32
  add), recomputes the argmax backpointers only along the surviving path
  (0.4% of device flops), does the traceback and the one-hot expansion.
All f32 arithmetic is bit-identical to the reference computation.
"""

import numpy as np

B, T, N = 128, 1024, 256
NCORES = 8
BL = B // NCORES          # 16 rows per core
GS = 4                    # rows per pipeline group
NG = BL // GS             # 4 groups
CH = 16                   # scan steps per potentials DMA chunk
NEG = -3.4e38             # max-reduce init; never wins against real scores

_CACHE = {}
TRACE = False          # test harness can enable NTFF tracing
_LAST_RESULTS = None   # BassKernelResults of the most recent device run


def _build(nbs, tm1):
    """Build the SPMD Bass program. nbs[t-1] = active rows at step t (1..tm1)."""
    from concourse import bacc, bass, tile

    mybir = bass.mybir
    f32 = mybir.dt.float32
    Alu = mybir.AluOpType
    Act = mybir.ActivationFunctionType

    nc = bacc.Bacc(None)
    transT_d = nc.declare_dram_parameter("transT", [128, 2 * N], f32, isOutput=False)
    pot_d = nc.declare_dram_parameter("pot", [BL, tm1, N], f32, isOutput=False)
    alpha0_d = nc.declare_dram_parameter("alpha0", [BL, N], f32, isOutput=False)
    ident_d = nc.declare_dram_parameter("ident", [128, 128], f32, isOutput=False)
    sel_d = nc.declare_dram_parameter("sel", [GS, GS * 128], f32, isOutput=False)
    mhist_d = nc.declare_dram_parameter(
        "mhist", [tm1, 128, BL, 2], f32, isOutput=True
    )

    with tile.TileContext(nc) as tc:
        with (
            tc.tile_pool(name="consts", bufs=1) as consts,
            tc.tile_pool(name="state", bufs=1) as state,
            tc.tile_pool(name="pots", bufs=2) as pots,
            tc.tile_pool(name="mall", bufs=3) as mall,
            tc.tile_pool(name="scrp", bufs=2) as scrp,
            tc.tile_pool(name="repp", bufs=3, space="PSUM") as repp,
            tc.tile_pool(name="mtp", bufs=2, space="PSUM") as mtp,
        ):
            transT = consts.tile([128, 2 * N], f32)
            nc.sync.dma_start(out=transT[:, :], in_=transT_d[:, :])
            ident = consts.tile([128, 128], f32)
            nc.sync.dma_start(out=ident[:, :], in_=ident_d[:, :])
            sel = consts.tile([GS, GS * 128], f32)
            nc.sync.dma_start(out=sel[:, :], in_=sel_d[:, :])

            alphaF = []
            for g in range(NG):
                t_ = state.tile([GS, N], f32, tag=f"alphaF{g}")
                nc.sync.dma_start(
                    out=t_[:, :], in_=alpha0_d[GS * g : GS * (g + 1), :]
                )
                alphaF.append(t_)

            reps = [None] * NG

            def emit_group_reps(g, nrows):
                # replicate alphaF[g] rows 0..nrows across all 128 partitions
                # (one rank-GS selector matmul per row, into one PSUM tile)
                rep = repp.tile([128, GS, N], f32, tag="rep")
                for r in range(nrows):
                    nc.tensor.matmul(
                        out=rep[:, r, :],
                        lhsT=sel[:, r * 128 : (r + 1) * 128],
                        rhs=alphaF[g][:, :],
                        start=True,
                        stop=True,
                    )
                reps[g] = rep

            nb1 = nbs[0] if nbs else 0
            for g in range((nb1 + GS - 1) // GS):
                emit_group_reps(g, min(GS, nb1 - GS * g))

            pot_sb = None
            cur_chunk = -1
            for t in range(1, tm1 + 1):
                nb = nbs[t - 1]
                if nb == 0:
                    continue
                ck = (t - 1) // CH
                if ck != cur_chunk:
                    cur_chunk = ck
                    c0 = ck * CH
                    cw = min(CH, tm1 - c0)
                    pot_sb = []
                    for g in range(NG):
                        pt = pots.tile([GS, CH, N], f32, tag=f"potc{g}")
                        nc.sync.dma_start(
                            out=pt[:, 0:cw, :],
                            in_=pot_d[GS * g : GS * (g + 1), c0 : c0 + cw, :],
                        )
                        pot_sb.append(pt)
                tcol = (t - 1) - ck * CH
                nb_next = nbs[t] if t < tm1 else 0
                m_all = mall.tile([128, BL, 2], f32, tag="mall")
                ga = (nb + GS - 1) // GS
                for g in range(ga):
                    b0 = GS * g
                    gsz = min(GS, nb - b0)
                    scr = scrp.tile([128, GS, 2, N], f32, tag="scr")
                    t_b = (
                        transT[:, :]
                        .rearrange("p (ti i) -> p ti i", ti=2)
                        .unsqueeze(1)
                        .broadcast_to((128, gsz, 2, N))
                    )
                    r_b = (
                        reps[g][:, 0:gsz, :]
                        .unsqueeze(2)
                        .broadcast_to((128, gsz, 2, N))
                    )
                    nc.vector.tensor_tensor(
                        out=scr[:, 0:gsz, :, :], in0=t_b, in1=r_b, op=Alu.add
                    )
                    nc.vector.tensor_reduce(
                        out=m_all[:, b0 : b0 + gsz, :],
                        in_=scr[:, 0:gsz, :, :],
                        axis=mybir.AxisListType.X,
                        op=Alu.max,
                    )
                    # group tail: transpose maxima + add potentials (PE), to SBUF
                    mT = mtp.tile([GS, 256], f32, tag="mT")
                    for ti in range(2):
                        nc.tensor.matmul(
                            out=mT[0:gsz, ti * 128 : (ti + 1) * 128],
                            lhsT=m_all[:, b0 : b0 + gsz, ti],
                            rhs=ident[:, :],
                            is_transpose=True,
                            start=True,
                            stop=False,
                        )
                        nc.tensor.matmul(
                            out=mT[0:gsz, ti * 128 : (ti + 1) * 128],
                            lhsT=ident[0:gsz, 0:gsz],
                            rhs=pot_sb[g][0:gsz, tcol, ti * 128 : (ti + 1) * 128],
                            start=False,
                            stop=True,
                        )
                    nc.scalar.activation(
                        out=alphaF[g][0:gsz, :], in_=mT[0:gsz, :], func=Act.Copy
                    )
                    nxt = min(GS, max(0, nb_next - b0))
                    if nxt > 0:
                        emit_group_reps(g, nxt)
                nc.sync.dma_start(
                    out=mhist_d[t - 1, :, 0:nb, :], in_=m_all[:, 0:nb, :]
                )
    nc.compile()
    return nc


def _get_program(nbs, tm1):
    key = (tuple(nbs), tm1)
    if key not in _CACHE:
        _CACHE[key] = _build(tuple(nbs), tm1)
    return _CACHE[key]


def _host_decode(pot, trans, lens, m_pre):
    """Traceback + one-hot on host. m_pre[t, b, j] valid for 1 <= t < len[b]."""
    Bs, Ts, Ns = pot.shape

    def alpha_at(t):
        if t == 0:
            return pot[:, 0, :]
        return m_pre[t] + pot[:, t, :]

    # frozen final alpha per row: alpha at t = len-1
    alpha_fin = np.empty((Bs, Ns), np.float32)
    for b in range(Bs):
        alpha_fin[b] = alpha_at(int(lens[b]) - 1)[b]
    last_tag = np.argmax(alpha_fin, axis=1).astype(np.int32)

    tags = np.zeros((Bs, Ts), np.int32)
    carry = last_tag.copy()
    transT = np.ascontiguousarray(trans.T)  # [next, prev]
    for t in range(Ts - 1, 0, -1):
        np.copyto(tags[:, t], np.where(t < lens, carry, 0))
        upd = t < lens
        if upd.any():
            a_prev = alpha_at(t - 1)                   # [B, N]
            sc = a_prev + transT[carry]                # [B, N] over prev i
            prev = np.argmax(sc, axis=1).astype(np.int32)
            carry = np.where(upd, prev, carry)
    tags[:, 0] = carry  # t=0 always < len (len >= 1)
    return tags


def kernel(potentials, transitions, sequence_lengths):
    from concourse.bass_utils import run_bass_kernel_spmd

    pot = np.ascontiguousarray(potentials, dtype=np.float32)
    trans = np.ascontiguousarray(transitions, dtype=np.float32)
    lens = np.asarray(sequence_lengths, dtype=np.int32)
    Bs, Ts, Ns = pot.shape
    tm1 = Ts - 1

    # deal rows (sorted by length desc) round-robin to cores
    order = np.argsort(-lens, kind="stable")
    core_rows = [order[c::NCORES] for c in range(NCORES)]
    # active-row count per step (same program for all cores): ceil(K_t / ncores)
    K = (lens[:, None] > np.arange(1, Ts)[None, :]).sum(axis=0)  # [tm1]
    nbs = tuple(int(-(-k // NCORES)) for k in K)

    nc = _get_program(nbs, tm1)

    transT_dev = np.empty((128, 2 * Ns), np.float32)
    for ti in range(2):
        # transT_dev[p, ti*N + i] = trans[i, ti*128 + p]
        transT_dev[:, ti * Ns : (ti + 1) * Ns] = trans[:, ti * 128 : (ti + 1) * 128].T
    ident = np.eye(128, dtype=np.float32)
    sel = np.zeros((GS, GS * 128), np.float32)
    for r in range(GS):
        sel[r, r * 128 : (r + 1) * 128] = 1.0

    in_maps = []
    for c in range(NCORES):
        rows = core_rows[c]
        in_maps.append(
            {
                "transT": transT_dev,
                "pot": np.ascontiguousarray(pot[rows, 1:, :]),
                "alpha0": np.ascontiguousarray(pot[rows, 0, :]),
                "ident": ident,
                "sel": sel,
            }
        )

    global _LAST_RESULTS
    res = run_bass_kernel_spmd(
        nc, in_maps, core_ids=list(range(NCORES)), trace=TRACE
    )
    _LAST_RESULTS = res

    # reassemble m_pre[t, b, j] (t >= 1)
    m_pre = np.zeros((Ts, Bs, Ns), np.float32)
    for c in range(NCORES):
        mh = res.results[c]["mhist"].reshape(tm1, 128, BL, 2)
        # mhist[t-1, p, lb, ti] = m_pre[t, rows[lb], ti*128 + p]
        m_pre[1:, core_rows[c], :] = (
            mh.transpose(0, 2, 3, 1).reshape(tm1, BL, Ns)
        )

    tags = _host_decode(pot, trans, lens, m_pre)
    out = np.eye(Ns, dtype=pot.dtype)[tags]
    return out


# revision 14
# speedup vs baseline: 1.1030x; 1.1030x over previous
"""Trainium2 Bass kernel for CRF Viterbi decode (nn_CRFLayer).

Strategy (pure data parallel over batch, per sharding hint):
- Host: sort batch rows by sequence length (desc), deal round-robin to the
  8 cores so every core gets an near-identical active-row schedule.
- Device (per core, 16 rows): the O(B*T*N^2) forward max-plus scan.
  Layout: scores_T[j_part, i_free] = transT[j, i] + alpha[i], computed as a
  single fused DVE tensor_tensor_reduce (add + max-reduce) per (row, j-tile).
  alpha (a 256-vector per row) is replicated across partitions by a rank-1
  PE matmul (ones x alpha) into PSUM; the per-step max outputs are
  transposed back to row-major by PE, the potentials added by an
  accumulating PE matmul (identity x pot), and ACT evicts PSUM->SBUF.
  Rows whose sequence ended are simply not computed (program is specialized
  on the active-row counts derived from sequence_lengths).
- Device streams out m_pre[t] = max_i(alpha_{t-1}[i] + trans[i, j]) per step.
- Host: reconstructs alpha_t = m_pre[t] + pot_t exactly (same single f32
  add), recomputes the argmax backpointers only along the surviving path
  (0.4% of device flops), does the traceback and the one-hot expansion.
All f32 arithmetic is bit-identical to the reference computation.
"""

import numpy as np

B, T, N = 128, 1024, 256
NCORES = 8
BL = B // NCORES          # 16 rows per core
GS = 4                    # rows per pipeline group
NG = BL // GS             # 4 groups
CH = 16                   # scan steps per potentials DMA chunk
NEG = -3.4e38             # max-reduce init; never wins against real scores

_CACHE = {}
TRACE = False          # test harness can enable NTFF tracing
_LAST_RESULTS = None   # BassKernelResults of the most recent device run


def _build(nbs, tm1):
    """Build the SPMD Bass program. nbs[t-1] = active rows at step t (1..tm1)."""
    from concourse import bacc, bass, tile

    mybir = bass.mybir
    f32 = mybir.dt.float32
    Alu = mybir.AluOpType
    Act = mybir.ActivationFunctionType

    nc = bacc.Bacc(None)
    transT_d = nc.declare_dram_parameter("transT", [128, 2 * N], f32, isOutput=False)
    pot_d = nc.declare_dram_parameter("pot", [BL, tm1, N], f32, isOutput=False)
    alpha0_d = nc.declare_dram_parameter("alpha0", [BL, N], f32, isOutput=False)
    ident_d = nc.declare_dram_parameter("ident", [128, 128], f32, isOutput=False)
    mhist_d = nc.declare_dram_parameter(
        "mhist", [tm1, 128, BL, 2], f32, isOutput=True
    )

    with tile.TileContext(nc) as tc:
        with (
            tc.tile_pool(name="consts", bufs=1) as consts,
            tc.tile_pool(name="state", bufs=1) as state,
            tc.tile_pool(name="pots", bufs=2) as pots,
            tc.tile_pool(name="mall", bufs=3) as mall,
            tc.tile_pool(name="scrp", bufs=2) as scrp,
            tc.tile_pool(name="repp", bufs=3) as repp,
            tc.tile_pool(name="a1p", bufs=2) as a1p,
            tc.tile_pool(name="mtp", bufs=2, space="PSUM") as mtp,
        ):
            transT = consts.tile([128, 2 * N], f32)
            nc.sync.dma_start(out=transT[:, :], in_=transT_d[:, :])
            ident = consts.tile([128, 128], f32)
            nc.sync.dma_start(out=ident[:, :], in_=ident_d[:, :])

            alphaF = []
            for g in range(NG):
                t_ = state.tile([GS, N], f32, tag=f"alphaF{g}")
                nc.sync.dma_start(
                    out=t_[:, :], in_=alpha0_d[GS * g : GS * (g + 1), :]
                )
                alphaF.append(t_)

            reps = [None] * NG

            def emit_group_reps(g, nrows):
                # collapse alphaF[g] onto one partition (DMA), then replicate
                # across all 128 partitions (gpsimd partition_broadcast)
                a1 = a1p.tile([1, GS * N], f32, tag="a1")
                nc.sync.dma_start(out=a1[0:1, :], in_=alphaF[g][:, :])
                rep = repp.tile([128, GS, N], f32, tag="rep")
                nc.gpsimd.partition_broadcast(
                    rep[:, :, :].rearrange("p r i -> p (r i)"), a1[0:1, :]
                )
                reps[g] = rep

            nb1 = nbs[0] if nbs else 0
            for g in range((nb1 + GS - 1) // GS):
                emit_group_reps(g, min(GS, nb1 - GS * g))

            pot_sb = None
            cur_chunk = -1
            for t in range(1, tm1 + 1):
                nb = nbs[t - 1]
                if nb == 0:
                    continue
                ck = (t - 1) // CH
                if ck != cur_chunk:
                    cur_chunk = ck
                    c0 = ck * CH
                    cw = min(CH, tm1 - c0)
                    pot_sb = []
                    for g in range(NG):
                        pt = pots.tile([GS, CH, N], f32, tag=f"potc{g}")
                        nc.sync.dma_start(
                            out=pt[:, 0:cw, :],
                            in_=pot_d[GS * g : GS * (g + 1), c0 : c0 + cw, :],
                        )
                        pot_sb.append(pt)
                tcol = (t - 1) - ck * CH
                nb_next = nbs[t] if t < tm1 else 0
                m_all = mall.tile([128, BL, 2], f32, tag="mall")
                ga = (nb + GS - 1) // GS
                for g in range(ga):
                    b0 = GS * g
                    gsz = min(GS, nb - b0)
                    scr = scrp.tile([128, GS, 2, N], f32, tag="scr")
                    t_b = (
                        transT[:, :]
                        .rearrange("p (ti i) -> p ti i", ti=2)
                        .unsqueeze(1)
                        .broadcast_to((128, gsz, 2, N))
                    )
                    r_b = (
                        reps[g][:, 0:gsz, :]
                        .unsqueeze(2)
                        .broadcast_to((128, gsz, 2, N))
                    )
                    nc.vector.tensor_tensor(
                        out=scr[:, 0:gsz, :, :], in0=t_b, in1=r_b, op=Alu.add
                    )
                    nc.vector.tensor_reduce(
                        out=m_all[:, b0 : b0 + gsz, :],
                        in_=scr[:, 0:gsz, :, :],
                        axis=mybir.AxisListType.X,
                        op=Alu.max,
                    )
                    # group tail: transpose maxima + add potentials (PE), to SBUF
                    mT = mtp.tile([GS, 256], f32, tag="mT")
                    for ti in range(2):
                        nc.tensor.matmul(
                            out=mT[0:gsz, ti * 128 : (ti + 1) * 128],
                            lhsT=m_all[:, b0 : b0 + gsz, ti],
                            rhs=ident[:, :],
                            is_transpose=True,
                            start=True,
                            stop=False,
                        )
                        nc.tensor.matmul(
                            out=mT[0:gsz, ti * 128 : (ti + 1) * 128],
                            lhsT=ident[0:gsz, 0:gsz],
                            rhs=pot_sb[g][0:gsz, tcol, ti * 128 : (ti + 1) * 128],
                            start=False,
                            stop=True,
                        )
                    nc.scalar.activation(
                        out=alphaF[g][0:gsz, :], in_=mT[0:gsz, :], func=Act.Copy
                    )
                    nxt = min(GS, max(0, nb_next - b0))
                    if nxt > 0:
                        emit_group_reps(g, nxt)
                nc.sync.dma_start(
                    out=mhist_d[t - 1, :, 0:nb, :], in_=m_all[:, 0:nb, :]
                )
    nc.compile()
    return nc


def _get_program(nbs, tm1):
    key = (tuple(nbs), tm1)
    if key not in _CACHE:
        _CACHE[key] = _build(tuple(nbs), tm1)
    return _CACHE[key]


def _host_decode(pot, trans, lens, m_pre):
    """Traceback + one-hot on host. m_pre[t, b, j] valid for 1 <= t < len[b]."""
    Bs, Ts, Ns = pot.shape

    def alpha_at(t):
        if t == 0:
            return pot[:, 0, :]
        return m_pre[t] + pot[:, t, :]

    # frozen final alpha per row: alpha at t = len-1
    alpha_fin = np.empty((Bs, Ns), np.float32)
    for b in range(Bs):
        alpha_fin[b] = alpha_at(int(lens[b]) - 1)[b]
    last_tag = np.argmax(alpha_fin, axis=1).astype(np.int32)

    tags = np.zeros((Bs, Ts), np.int32)
    carry = last_tag.copy()
    transT = np.ascontiguousarray(trans.T)  # [next, prev]
    for t in range(Ts - 1, 0, -1):
        np.copyto(tags[:, t], np.where(t < lens, carry, 0))
        upd = t < lens
        if upd.any():
            a_prev = alpha_at(t - 1)                   # [B, N]
            sc = a_prev + transT[carry]                # [B, N] over prev i
            prev = np.argmax(sc, axis=1).astype(np.int32)
            carry = np.where(upd, prev, carry)
    tags[:, 0] = carry  # t=0 always < len (len >= 1)
    return tags


def kernel(potentials, transitions, sequence_lengths):
    from concourse.bass_utils import run_bass_kernel_spmd

    pot = np.ascontiguousarray(potentials, dtype=np.float32)
    trans = np.ascontiguousarray(transitions, dtype=np.float32)
    lens = np.asarray(sequence_lengths, dtype=np.int32)
    Bs, Ts, Ns = pot.shape
    tm1 = Ts - 1

    # deal rows (sorted by length desc) round-robin to cores
    order = np.argsort(-lens, kind="stable")
    core_rows = [order[c::NCORES] for c in range(NCORES)]
    # active-row count per step (same program for all cores): ceil(K_t / ncores)
    K = (lens[:, None] > np.arange(1, Ts)[None, :]).sum(axis=0)  # [tm1]
    nbs = tuple(int(-(-k // NCORES)) for k in K)

    nc = _get_program(nbs, tm1)

    transT_dev = np.empty((128, 2 * Ns), np.float32)
    for ti in range(2):
        # transT_dev[p, ti*N + i] = trans[i, ti*128 + p]
        transT_dev[:, ti * Ns : (ti + 1) * Ns] = trans[:, ti * 128 : (ti + 1) * 128].T
    ident = np.eye(128, dtype=np.float32)

    in_maps = []
    for c in range(NCORES):
        rows = core_rows[c]
        in_maps.append(
            {
                "transT": transT_dev,
                "pot": np.ascontiguousarray(pot[rows, 1:, :]),
                "alpha0": np.ascontiguousarray(pot[rows, 0, :]),
                "ident": ident,
            }
        )

    global _LAST_RESULTS
    res = run_bass_kernel_spmd(
        nc, in_maps, core_ids=list(range(NCORES)), trace=TRACE
    )
    _LAST_RESULTS = res

    # reassemble m_pre[t, b, j] (t >= 1)
    m_pre = np.zeros((Ts, Bs, Ns), np.float32)
    for c in range(NCORES):
        mh = res.results[c]["mhist"].reshape(tm1, 128, BL, 2)
        # mhist[t-1, p, lb, ti] = m_pre[t, rows[lb], ti*128 + p]
        m_pre[1:, core_rows[c], :] = (
            mh.transpose(0, 2, 3, 1).reshape(tm1, BL, Ns)
        )

    tags = _host_decode(pot, trans, lens, m_pre)
    out = np.eye(Ns, dtype=pot.dtype)[tags]
    return out
